# revision 1
# baseline (speedup 1.0000x reference)
"""Multi-head causal self-attention (B=4, N=2048, D=1024, H=16) on 8 TRN2 cores.

Sharding: 8 cores = 4 batches x 2 head-groups (8 heads / 512 dims each).
Per core (batch b, group g):
  - QKV projections computed in transposed layout (dims on partitions):
      Q^T, K^T = W^T-chunks (lhsT) x x^T (rhs), accumulated over 8 din chunks.
      V computed in natural [token, dv] layout (lhsT = x^T chunk).
  - Attention computed as S^T tiles [keys(128) x queries(512)] so that
    exp(S) feeds the P^T.V matmul directly (contraction over keys on
    partitions, no transposes anywhere). Softmax denominators come from a
    ones-column appended to V (row HD of the PV accumulator); normalization
    is deferred and batched per strip. Causal masking = skip blocks above
    the diagonal + multiply diagonal-region tiles by precomputed 0/1 masks
    after exp. No max-subtraction: scores are ~N(0,1) after the 1/sqrt(hd)
    scale, exp is safe in fp32.
  - O-projection partial: attnT (lhsT) x Wo-slice (rhs) -> [2048, 1024]
    partial output per core; host sums the two group partials per batch.

Dtypes: scores path float32r (fp32 storage, ~tf32 matmul precision, full PE
rate); P/V path bfloat16 (probs in [0,1], V ~N(0,1)).
"""

import numpy as np
import ml_dtypes

import concourse.bass as bass
import concourse.tile as tile
from concourse import bacc, mybir
from concourse import bass_utils
from concourse._compat import with_exitstack
from concourse.bass import ts, ds

B, N, D, H, HD = 4, 2048, 1024, 16, 64
GROUPS = 2              # head groups (cores per batch)
DC = D // GROUPS        # 512 dims per core
HPC = H // GROUPS       # 8 heads per core
P = 128
QW = 512                # query strip width / matmul free dim
NDIN = D // P           # 8 contraction chunks for QKV
NSTRIP = DC // P        # 4 dq strips per core (2 heads each)
NTT = N // P            # 16 token tiles
NTS = N // QW           # 4 token strips
NQB = QW // P           # 4 query blocks per strip

F32 = mybir.dt.float32
F32R = mybir.dt.float32r
BF16 = mybir.dt.bfloat16


def _emit(ctx, tc, xT, wq, wk, wv, wo, bq, bk, bv, masks, out):
    nc = tc.nc
    EXP = mybir.ActivationFunctionType.Exp

    const = ctx.enter_context(tc.tile_pool(name="const", bufs=1))
    p_mm = ctx.enter_context(tc.tile_pool(name="p_mm", bufs=2, space="PSUM"))
    p_pt = ctx.enter_context(tc.tile_pool(name="p_pt", bufs=3))
    p_small = ctx.enter_context(tc.tile_pool(name="p_small", bufs=2))
    p_dram = ctx.enter_context(tc.tile_pool(name="p_dram", bufs=2, space="DRAM"))

    # constants on the GpSimd (SWDGE) queue so they don't serialize with the
    # x^T stream on the sync (HWDGE) queue. maskt = one triangular 0/1 tile.
    maskt = const.tile([P, P], BF16)
    nc.gpsimd.dma_start(out=maskt, in_=masks)
    bqt = const.tile([P, NSTRIP], F32)
    nc.gpsimd.dma_start(out=bqt, in_=bq.rearrange("(s p) -> p s", p=P))
    bkt = const.tile([P, NSTRIP], F32)
    nc.gpsimd.dma_start(out=bkt, in_=bk.rearrange("(s p) -> p s", p=P))
    bvb = const.tile([P, DC], F32)
    nc.gpsimd.dma_start(out=bvb, in_=bv.unsqueeze(0).partition_broadcast(P))

    # persistent per-batch tensors
    attnT = const.tile([P, NSTRIP, N], BF16)                # normalized attn^T
    vplus = const.tile([P, NTT, HPC, HD + 1], BF16)         # V | ones column
    # memset on an f32r/bf16 matmul-input tile is invalid ISA; write the ones
    # column via a DVE copy from an f32 staging tile (a valid rounding producer)
    ones_f32 = const.tile([P, NTT * HPC], F32)
    nc.vector.memset(ones_f32, 1.0)
    nc.vector.tensor_copy(
        out=vplus[:, :, :, HD:HD + 1],
        in_=ones_f32.rearrange("p (a b) -> p a b", b=HPC).unsqueeze(3),
    )

    # Wo loaded up-front so phase C never waits on its DMA
    wor = wo.rearrange("(c p) f -> c p f", p=P)
    wot = const.tile([P, NSTRIP, D], BF16)
    for c in range(NSTRIP):
        nc.gpsimd.dma_start(out=wot[:, c, :], in_=wor[c])

    wqr = wq.rearrange("(c p) f -> c p f", p=P)
    wkr = wk.rearrange("(c p) f -> c p f", p=P)
    wvr = wv.rearrange("(c p) f -> c p f", p=P)
    xTr = xT.rearrange("(c p) n -> c p n", p=P)

    with tc.tile_pool(name="p_xt", bufs=1) as p_xt:
        xt = p_xt.tile([P, NDIN, N], BF16)          # x^T resident, 64KB/part
        # token-strip-major loads so early-strip compute can start ASAP
        for t in range(NTS):
            for c in range(NDIN):
                nc.sync.dma_start(
                    out=xt[:, c, ts(t, QW)], in_=xTr[c, :, ts(t, QW)])

        with (
            tc.tile_pool(name="p_w", bufs=2) as p_w,
            tc.tile_pool(name="p_qk", bufs=2) as p_qk,
            tc.tile_pool(name="p_st", bufs=2, space="PSUM") as p_st,
            tc.tile_pool(name="p_pv", bufs=2, space="PSUM") as p_pv,
        ):
            def attn_group(s, h2, qs, qts, kts, sums_sb):
                """S^T/exp/PV for one (head, query strip).

                Work units: full-width kc pairs below the diagonal region,
                then two packed diagonal units with shrinking query widths
                (512+384 and 256+128) — queries before the key block are
                skipped entirely, the remaining 128-wide leading wedge of
                each unit gets the triangular mask.
                """
                po = h2 * HD
                h = 2 * s + h2
                nfull = NQB * qs             # unmasked key blocks 0..nfull-1
                nkc = nfull + NQB
                q0 = qs * QW
                pvp = p_pv.tile([HD + 1, QW], F32, tag="pv", name="pvp")

                units = []
                for ip in range(nfull // 2):
                    units.append(("full", ip))
                units.append(("diagA", None))
                units.append(("diagB", None))

                def emit_s(unit):
                    kind, ip = unit
                    if kind == "full":
                        pst = p_st.tile([P, 2, QW], F32, tag="st", name="pst")
                        for j2 in range(2):
                            kc = 2 * ip + j2
                            nc.tensor.matmul(
                                pst[:, j2, :],
                                lhsT=kts[po:po + HD, ts(kc, P)],
                                rhs=qts[po:po + HD, ts(qs, QW)],
                                start=True, stop=True,
                            )
                        pt = p_pt.tile([P, 2, QW], BF16, tag="pt", name="pt")
                        nc.scalar.activation(out=pt, in_=pst, func=EXP, scale=0.125)
                        return pt
                    if kind == "diagA":
                        # j=0: kc=nfull,   queries [0:512), tri on cols 0:128
                        # j=1: kc=nfull+1, queries [128:512), tri on cols 0:128
                        pst = p_st.tile([P, 2, QW], F32, tag="st", name="pst")
                        nc.tensor.matmul(
                            pst[:, 0, :],
                            lhsT=kts[po:po + HD, ts(nfull, P)],
                            rhs=qts[po:po + HD, ts(qs, QW)],
                            start=True, stop=True,
                        )
                        nc.tensor.matmul(
                            pst[:, 1, 0:3 * P],
                            lhsT=kts[po:po + HD, ts(nfull + 1, P)],
                            rhs=qts[po:po + HD, ds(q0 + P, 3 * P)],
                            start=True, stop=True,
                        )
                        pt = p_pt.tile([P, 2, QW], BF16, tag="pt", name="pt")
                        nc.scalar.activation(out=pt, in_=pst, func=EXP, scale=0.125)
                        nc.vector.tensor_mul(pt[:, 0, 0:P], pt[:, 0, 0:P], maskt)
                        nc.vector.tensor_mul(pt[:, 1, 0:P], pt[:, 1, 0:P], maskt)
                        return pt
                    # diagB: j=2: kc=nfull+2, queries [256:512) at cols 0:256;
                    #        j=3: kc=nfull+3, queries [384:512) at cols 256:384
                    pst = p_st.tile([P, QW], F32, tag="st", name="pst")
                    nc.tensor.matmul(
                        pst[:, 0:2 * P],
                        lhsT=kts[po:po + HD, ts(nfull + 2, P)],
                        rhs=qts[po:po + HD, ds(q0 + 2 * P, 2 * P)],
                        start=True, stop=True,
                    )
                    nc.tensor.matmul(
                        pst[:, 2 * P:3 * P],
                        lhsT=kts[po:po + HD, ts(nfull + 3, P)],
                        rhs=qts[po:po + HD, ds(q0 + 3 * P, P)],
                        start=True, stop=True,
                    )
                    pt = p_pt.tile([P, QW], BF16, tag="pt", name="pt")
                    nc.scalar.activation(out=pt, in_=pst, func=EXP, scale=0.125)
                    nc.vector.tensor_mul(pt[:, 0:P], pt[:, 0:P], maskt)
                    nc.vector.tensor_mul(pt[:, 2 * P:3 * P], pt[:, 2 * P:3 * P], maskt)
                    return pt

                def emit_pv(unit, pt):
                    kind, ip = unit
                    if kind == "full":
                        for j2 in range(2):
                            kc = 2 * ip + j2
                            nc.tensor.matmul(
                                pvp, lhsT=vplus[:, kc, h, :], rhs=pt[:, j2, :],
                                start=(kc == 0), stop=False,
                            )
                    elif kind == "diagA":
                        nc.tensor.matmul(
                            pvp, lhsT=vplus[:, nfull, h, :], rhs=pt[:, 0, :],
                            start=(nfull == 0), stop=False,
                        )
                        nc.tensor.matmul(
                            pvp[:, P:4 * P], lhsT=vplus[:, nfull + 1, h, :],
                            rhs=pt[:, 1, 0:3 * P], start=False, stop=False,
                        )
                    else:
                        nc.tensor.matmul(
                            pvp[:, 2 * P:4 * P], lhsT=vplus[:, nfull + 2, h, :],
                            rhs=pt[:, 0:2 * P], start=False, stop=False,
                        )
                        nc.tensor.matmul(
                            pvp[:, 3 * P:4 * P], lhsT=vplus[:, nfull + 3, h, :],
                            rhs=pt[:, 2 * P:3 * P], start=False, stop=True,
                        )

                LOOKP = 1
                pts = {}
                for i in range(len(units) + LOOKP):
                    if i < len(units):
                        pts[i] = emit_s(units[i])
                    if i >= LOOKP:
                        j = i - LOOKP
                        emit_pv(units[j], pts.pop(j))
                nc.vector.tensor_copy(
                    out=sums_sb[32 * qs:32 * qs + 1, h2, :],
                    in_=pvp[HD:HD + 1, :])
                nc.vector.tensor_copy(
                    out=attnT[po:po + HD, s, ts(qs, QW)], in_=pvp[0:HD, :])

            def normalize_h2(s, h2, sums_sb):
                """Batched softmax normalization for one head (4 query strips).

                1/s computed as exp(-ln(s)) on the Scalar engine and the
                normalize multiplies run on GpSimd: a 3.3us DVE RECIPROCAL
                head-of-line blocks the mask-muls the next head's PV matmuls
                depend on, stalling the PE ~2-3us per head.
                """
                po = h2 * HD
                lns = p_small.tile([P, QW], F32, tag="lns", name="lns")
                nc.scalar.activation(out=lns, in_=sums_sb[:, h2, :],
                                     func=mybir.ActivationFunctionType.Ln)
                recip_sb = p_small.tile([P, QW], F32, tag="recip", name="recip_sb")
                nc.scalar.activation(out=recip_sb, in_=lns, func=EXP, scale=-1.0)
                # broadcast across partitions via a DRAM round-trip
                # (SBUF-source partition-broadcast DMA is rejected)
                recip_d = p_dram.tile([NTS, QW], F32, tag="recipd", name="recip_d")
                nc.sync.dma_start(
                    out=recip_d,
                    in_=recip_sb.rearrange("(a b) f -> a b f", b=32)[:, 0, :])
                for qs in range(NTS):
                    # full-128-partition broadcast so rb[po:po+HD] shares
                    # the base partition with the attnT slice (DVE rule)
                    rb = p_small.tile([P, QW], F32, tag="rb", bufs=3, name="rb")
                    nc.sync.dma_start(
                        out=rb,
                        in_=recip_d[qs, :].unsqueeze(0).partition_broadcast(P))
                    sl = attnT[po:po + HD, s, ts(qs, QW)]
                    nc.vector.tensor_mul(out=sl, in0=sl, in1=rb[po:po + HD, :])

            pending = [None]
            for s in range(NSTRIP):
                wqs = p_w.tile([P, NDIN, P], BF16, tag="wq")
                wks = p_w.tile([P, NDIN, P], BF16, tag="wk")
                for c in range(NDIN):
                    nc.gpsimd.dma_start(out=wqs[:, c, :], in_=wqr[c, :, ts(s, P)])
                    nc.gpsimd.dma_start(out=wks[:, c, :], in_=wkr[c, :, ts(s, P)])
                qts = p_qk.tile([P, N], BF16, tag="qt")
                kts = p_qk.tile([P, N], BF16, tag="kt")
                # sums rows at partition offsets {0,32,64,96} x 2 col blocks
                # (DVE partition offsets must be 32-aligned); unused rows are
                # memset to 1.0 so the batched reciprocal stays finite
                sums_sb = p_small.tile([P, 2, QW], F32, tag="sums")
                nc.gpsimd.memset(sums_sb, 1.0)
                for t in range(NTS):
                    psq = p_mm.tile([P, QW], F32, tag="mm", name="psq")
                    for c in range(NDIN):
                        nc.tensor.matmul(
                            psq, lhsT=wqs[:, c, :], rhs=xt[:, c, ts(t, QW)],
                            start=(c == 0), stop=(c == NDIN - 1),
                        )
                    nc.vector.tensor_scalar_add(
                        out=qts[:, ts(t, QW)], in0=psq, scalar1=bqt[:, s:s + 1])
                    psk = p_mm.tile([P, QW], F32, tag="mm", name="psk")
                    for c in range(NDIN):
                        nc.tensor.matmul(
                            psk, lhsT=wks[:, c, :], rhs=xt[:, c, ts(t, QW)],
                            start=(c == 0), stop=(c == NDIN - 1),
                        )
                    nc.vector.tensor_scalar_add(
                        out=kts[:, ts(t, QW)], in0=psk, scalar1=bkt[:, s:s + 1])

                    if s == 0 and t == 0:
                        # V = x @ Wv + bv for all heads, nested here so the
                        # strip-0 attention (which needs early V tiles) can
                        # start as soon as possible
                        with tc.tile_pool(name="p_wv", bufs=1) as p_wv:
                            wvt = p_wv.tile([P, NDIN, DC], BF16)
                            for c in range(NDIN):
                                nc.gpsimd.dma_start(out=wvt[:, c, :], in_=wvr[c])
                            for tt in range(NTT):
                                psv = p_mm.tile([P, DC], F32, tag="mm", name="psv")
                                for c in range(NDIN):
                                    nc.tensor.matmul(
                                        psv, lhsT=xt[:, c, ts(tt, P)],
                                        rhs=wvt[:, c, :],
                                        start=(c == 0), stop=(c == NDIN - 1),
                                    )
                                nc.vector.tensor_add(
                                    out=vplus[:, tt, :, 0:HD],
                                    in0=psv.rearrange("p (h d) -> p h d", d=HD),
                                    in1=bvb.rearrange("p (h d) -> p h d", d=HD),
                                )

                    if s == 0:
                        # strip 0: interleave attention with projections so
                        # compute starts before all x^T strips have landed
                        attn_group(s, 0, t, qts, kts, sums_sb)
                        attn_group(s, 1, t, qts, kts, sums_sb)
                        if t == 1 and pending[0] is not None:
                            pending[0](); pending[0] = None
                if s == 0:
                    pending[0] = (lambda ss=s, sb=sums_sb:
                                  (normalize_h2(ss, 0, sb),
                                   normalize_h2(ss, 1, sb)))
                else:
                    # head-major; each head's normalization is emitted after
                    # the NEXT head's first groups so it backfills engine idle
                    # slots instead of head-of-line blocking the critical chain
                    for h2 in range(2):
                        for qs in range(NTS):
                            attn_group(s, h2, qs, qts, kts, sums_sb)
                            if qs == 1 and pending[0] is not None:
                                pending[0](); pending[0] = None
                        pending[0] = (lambda ss=s, hh=h2, sb=sums_sb:
                                      normalize_h2(ss, hh, sb))

            if pending[0] is not None:
                pending[0](); pending[0] = None

    # ---- phase C: partial output = attnT^T @ Wo_slice ----
    with (
        tc.tile_pool(name="p_osb", bufs=3) as p_osb,
        tc.tile_pool(name="p_c", bufs=4, space="PSUM") as p_c,
    ):
        # software pipeline: emit chunks 0..2 of group g before the last
        # chunk + eviction of group g-LOOKC, so the PE has runway while the
        # final strip's normalization lands
        LOOKC = 3
        groups = [(tt, half) for tt in range(NTT) for half in range(2)]
        psos = {}
        osbs = {}
        for g in range(len(groups) + LOOKC):
            if g < len(groups):
                tt, half = groups[g]
                pso = p_c.tile([P, QW], F32, tag="c", name="pso")
                for c in range(NSTRIP - 1):
                    nc.tensor.matmul(
                        pso, lhsT=attnT[:, c, ts(tt, P)],
                        rhs=wot[:, c, ds(half * QW, QW)],
                        start=(c == 0), stop=False,
                    )
                psos[g] = pso
            if g >= LOOKC:
                tt, half = groups[g - LOOKC]
                pso = psos.pop(g - LOOKC)
                c = NSTRIP - 1
                nc.tensor.matmul(
                    pso, lhsT=attnT[:, c, ts(tt, P)],
                    rhs=wot[:, c, ds(half * QW, QW)],
                    start=False, stop=True,
                )
                if half == 0:
                    osbs[tt] = p_osb.tile([P, D], F32, tag="osb", name="osb")
                nc.vector.tensor_copy(
                    out=osbs[tt][:, ds(half * QW, QW)], in_=pso)
                if half == 1:
                    nc.sync.dma_start(out=out[ts(tt, P), :], in_=osbs.pop(tt))


_emit_wrapped = with_exitstack(_emit)

_NC_CACHE = None


def _build():
    global _NC_CACHE
    if _NC_CACHE is not None:
        return _NC_CACHE
    nc = bacc.Bacc("TRN2", target_bir_lowering=False, debug=False)
    xT = nc.dram_tensor("xt", [D, N], BF16, kind="ExternalInput").ap()
    wq = nc.dram_tensor("wq", [D, DC], BF16, kind="ExternalInput").ap()
    wk = nc.dram_tensor("wk", [D, DC], BF16, kind="ExternalInput").ap()
    wv = nc.dram_tensor("wv", [D, DC], BF16, kind="ExternalInput").ap()
    wo = nc.dram_tensor("wo", [DC, D], BF16, kind="ExternalInput").ap()
    bq = nc.dram_tensor("bq", [DC], F32, kind="ExternalInput").ap()
    bk = nc.dram_tensor("bk", [DC], F32, kind="ExternalInput").ap()
    bv = nc.dram_tensor("bv", [DC], F32, kind="ExternalInput").ap()
    masks = nc.dram_tensor("masks", [P, P], BF16, kind="ExternalInput").ap()
    out = nc.dram_tensor("out", [N, D], F32, kind="ExternalOutput").ap()
    with tile.TileContext(nc) as tc:
        _emit_wrapped(tc, xT, wq, wk, wv, wo, bq, bk, bv, masks, out)
    nc.compile()
    _NC_CACHE = nc
    return nc


def _make_masks():
    # triangular 0/1 tile for the diagonal blocks of S^T: key <= query kept
    return np.triu(np.ones((P, P), np.float32)).astype(ml_dtypes.bfloat16)


def _in_maps(x, Wq, bq, Wk, bk, Wv, bv, Wo):
    masks = _make_masks()
    maps = []
    for b in range(B):
        xt_b = np.ascontiguousarray(np.asarray(x[b]).T)
        for g in range(GROUPS):
            sl = slice(g * DC, (g + 1) * DC)
            bf = ml_dtypes.bfloat16
            maps.append({
                "xt": xt_b.astype(bf),
                "wq": np.ascontiguousarray(Wq[:, sl]).astype(bf),
                "wk": np.ascontiguousarray(Wk[:, sl]).astype(bf),
                "wv": np.ascontiguousarray(Wv[:, sl]).astype(bf),
                "wo": np.ascontiguousarray(Wo[sl, :]).astype(bf),
                "bq": np.ascontiguousarray(bq[sl]),
                "bk": np.ascontiguousarray(bk[sl]),
                "bv": np.ascontiguousarray(bv[sl]),
                "masks": masks,
            })
    return maps


def run(inputs, trace=False, tmpdir=None):
    """Build+run on 8 cores. Returns (out [B,N,D] f32, BassKernelResults)."""
    x = np.asarray(inputs["x"], np.float32)
    args = [np.asarray(inputs[k], np.float32) for k in
            ("Wq", "bq", "Wk", "bk", "Wv", "bv", "Wo")]
    bo = np.asarray(inputs["bo"], np.float32)
    nc = _build()
    maps = _in_maps(x, *args)
    if trace:
        bass_utils.upload_artifacts = lambda d: d
    res = bass_utils.run_bass_kernel_spmd(
        nc, maps, core_ids=list(range(8)), trace=trace, tmpdir=tmpdir)
    out = np.empty((B, N, D), np.float32)
    for b in range(B):
        out[b] = res.results[2 * b]["out"] + res.results[2 * b + 1]["out"] + bo
    return out, res


def kernel(**inputs):
    out, _ = run(inputs)
    return out



# revision 3
# speedup vs baseline: 1.1085x; 1.1085x over previous
"""Multi-head causal self-attention (B=4, N=2048, D=1024, H=16) on 8 TRN2 cores.

Sharding: 8 cores = 4 batches x 2 head-groups (8 heads / 512 dims each).

v2 schedule (qs-major): the outer loop walks query strips (groups g=0..3);
for each group all 4 head-pairs run their S^T/exp/PV units. The two heads of
a pair live on SBUF partitions 0-63 / 64-127, so their S^T matmuls (K=64
contraction) are emitted back-to-back and execute CONCURRENTLY in the two
row-halves of the PE array (row tiling, tile_position (0,0)/(64,0) inferred
from base partitions). The attention stream is scalar-engine(exp)-bound, so
PE filler work is interleaved between units:
  - QKV projection chunks for token strip g+1 (needed by group g+1),
  - O-projection chunks for query strip g-1 (normalized at group-g start),
  - softmax-normalization multiplies for group g-1.
Softmax denominators ride as a ones-column in V (PV row HD); reciprocals are
computed with the custom-DVE reciprocal_approx_fast (no ACT table switches -
the scalar engine runs Exp only, one table load for the whole kernel).
Causal masking = skip blocks above the diagonal; the four diagonal-region
blocks per (pair, group) shrink 512/384/256/128 wide, exp'd at exact width,
with a precomputed 0/1 triangle multiplied onto the leading 128 columns.

Dtypes: scores fp32 PSUM -> exp -> bf16 P^T; V/P^T path bf16; attnT bf16;
O-partials f32; host sums the two group partials per batch.
"""

import numpy as np
import ml_dtypes

import concourse.bass as bass
import concourse.tile as tile
from concourse import bacc, mybir
from concourse import bass_utils
from concourse._compat import with_exitstack
from concourse.bass import ts, ds

B, N, D, H, HD = 4, 2048, 1024, 16, 64
GROUPS = 2              # head groups (cores per batch)
DC = D // GROUPS        # 512 dims per core
HPC = H // GROUPS       # 8 heads per core
P = 128
QW = 512                # query strip width / matmul free dim
NDIN = D // P           # 8 contraction chunks for QKV
NPAIR = DC // P         # 4 head-pairs (dq strips) per core
NTT = N // P            # 16 token tiles
NTS = N // QW           # 4 token strips == query groups
NQB = QW // P           # 4 key blocks per token strip

F32 = mybir.dt.float32
BF16 = mybir.dt.bfloat16


def _emit(ctx, tc, xT, wq, wk, wv, wo, bq, bk, bv, masks, out):
    nc = tc.nc
    EXP = mybir.ActivationFunctionType.Exp

    const = ctx.enter_context(tc.tile_pool(name="const", bufs=1))
    p_mm = ctx.enter_context(tc.tile_pool(name="p_mm", bufs=2, space="PSUM"))
    p_st = ctx.enter_context(tc.tile_pool(name="p_st", bufs=2, space="PSUM"))
    p_pv = ctx.enter_context(tc.tile_pool(name="p_pv", bufs=1, space="PSUM"))
    p_pt = ctx.enter_context(tc.tile_pool(name="p_pt", bufs=3))
    p_sm = ctx.enter_context(tc.tile_pool(name="p_sm", bufs=2))
    p_osb = ctx.enter_context(tc.tile_pool(name="p_osb", bufs=3))
    p_dram = ctx.enter_context(tc.tile_pool(name="p_dram", bufs=2, space="DRAM"))

    # ---- constants / persistent tensors ----
    maskt = const.tile([P, P], BF16)
    nc.gpsimd.dma_start(out=maskt, in_=masks)
    bqt = const.tile([P, NPAIR], F32)
    nc.gpsimd.dma_start(out=bqt, in_=bq.rearrange("(s p) -> p s", p=P))
    bkt = const.tile([P, NPAIR], F32)
    nc.gpsimd.dma_start(out=bkt, in_=bk.rearrange("(s p) -> p s", p=P))
    bvb = const.tile([P, DC], F32)
    nc.gpsimd.dma_start(out=bvb, in_=bv.unsqueeze(0).partition_broadcast(P))

    attnT = const.tile([P, NPAIR, N], BF16)           # unnormalized attn^T
    vplus = const.tile([P, NTT, HPC, HD + 1], BF16)   # V | ones column
    # memset on a bf16 matmul-input tile is invalid ISA; write the ones
    # column via a DVE copy from an f32 staging tile
    ones_f32 = const.tile([P, NTT * HPC], F32)
    nc.vector.memset(ones_f32, 1.0)
    nc.vector.tensor_copy(
        out=vplus[:, :, :, HD:HD + 1],
        in_=ones_f32.rearrange("p (a b) -> p a b", b=HPC).unsqueeze(3),
    )

    # softmax denominators: partition row 32*pair, free [group, h2, q]
    sums_all = const.tile([P, NTS, 2, QW], F32)
    nc.gpsimd.memset(sums_all, 1.0)

    # weights resident for the whole kernel (gpsimd/SWDGE queue)
    wqr = wq.rearrange("(c p) f -> c p f", p=P)
    wkr = wk.rearrange("(c p) f -> c p f", p=P)
    wvr = wv.rearrange("(c p) f -> c p f", p=P)
    wor = wo.rearrange("(c p) f -> c p f", p=P)
    wqs = const.tile([P, NPAIR, NDIN, P], BF16)
    wks = const.tile([P, NPAIR, NDIN, P], BF16)
    for s in range(NPAIR):
        for c in range(NDIN):
            nc.gpsimd.dma_start(out=wqs[:, s, c, :], in_=wqr[c, :, ts(s, P)])
            nc.gpsimd.dma_start(out=wks[:, s, c, :], in_=wkr[c, :, ts(s, P)])
    wvt = const.tile([P, NDIN, DC], BF16)
    for c in range(NDIN):
        nc.gpsimd.dma_start(out=wvt[:, c, :], in_=wvr[c])
    wot = const.tile([P, NPAIR, D], BF16)
    for c in range(NPAIR):
        nc.gpsimd.dma_start(out=wot[:, c, :], in_=wor[c])

    # x^T resident, token-strip-major loads (sync/HWDGE queue)
    xTr = xT.rearrange("(c p) n -> c p n", p=P)
    xt = const.tile([P, NDIN, N], BF16)
    for t in range(NTS):
        for c in range(NDIN):
            nc.sync.dma_start(out=xt[:, c, ts(t, QW)], in_=xTr[c, :, ts(t, QW)])

    qts = const.tile([P, NPAIR, N], BF16)
    kts = const.tile([P, NPAIR, N], BF16)

    # ---- PE filler generator: projections for token strip t ----
    def proj_steps(t):
        """Yield emission closures; each emits ~1-2 matmuls (one PE slot)."""
        for s in range(NPAIR):
            for which in range(2):  # 0 = Q, 1 = K
                wtile = wqs if which == 0 else wks
                btile = bqt if which == 0 else bkt
                dst = qts if which == 0 else kts
                ps = p_mm.tile([P, QW], F32, tag="mm", name="ps_proj")
                for c0 in range(0, NDIN, 2):
                    def step(ps=ps, s=s, c0=c0, wtile=wtile, t=t):
                        for c in (c0, c0 + 1):
                            nc.tensor.matmul(
                                ps, lhsT=wtile[:, s, c, :],
                                rhs=xt[:, c, ts(t, QW)],
                                start=(c == 0), stop=(c == NDIN - 1),
                            )
                    yield step
                def fin(ps=ps, s=s, t=t, btile=btile, dst=dst):
                    nc.vector.tensor_scalar_add(
                        out=dst[:, s, ts(t, QW)], in0=ps,
                        scalar1=btile[:, s:s + 1])
                yield fin
        for tt in range(NQB * t, NQB * (t + 1)):
            psv = p_mm.tile([P, DC], F32, tag="mm", name="psv")
            for c0 in range(0, NDIN, 2):
                def step(psv=psv, tt=tt, c0=c0):
                    for c in (c0, c0 + 1):
                        nc.tensor.matmul(
                            psv, lhsT=xt[:, c, ts(tt, P)], rhs=wvt[:, c, :],
                            start=(c == 0), stop=(c == NDIN - 1),
                        )
                yield step
            def finv(psv=psv, tt=tt):
                nc.vector.tensor_add(
                    out=vplus[:, tt, :, 0:HD],
                    in0=psv.rearrange("p (h d) -> p h d", d=HD),
                    in1=bvb.rearrange("p (h d) -> p h d", d=HD),
                )
            yield finv

    # ---- PE filler generator: O-projection for query strip b ----
    def o_steps(b):
        for tt in range(NQB * b, NQB * (b + 1)):
            osb = p_osb.tile([P, D], F32, tag="osb", name="osb")
            for half in range(2):
                pso = p_mm.tile([P, QW], F32, tag="mm", name="pso")
                for c0 in range(0, NPAIR, 2):
                    def step(pso=pso, tt=tt, half=half, c0=c0):
                        for c in (c0, c0 + 1):
                            nc.tensor.matmul(
                                pso, lhsT=attnT[:, c, ts(tt, P)],
                                rhs=wot[:, c, ds(half * QW, QW)],
                                start=(c == 0), stop=(c == NPAIR - 1),
                            )
                    yield step
                def fino(pso=pso, osb=osb, tt=tt, half=half):
                    nc.vector.tensor_copy(
                        out=osb[:, ds(half * QW, QW)], in_=pso)
                    if half == 1:
                        nc.sync.dma_start(out=out[ts(tt, P), :], in_=osb)
                yield fino

    # ---- normalization for group g (runs inside group g+1's stream) ----
    def norm_steps(g):
        recip = p_sm.tile([P, 2, QW], F32, tag="recip", name="recip")
        def r0():
            nc.vector.reciprocal_approx_fast(
                out=recip[:, 0, :], in_=sums_all[:, g, 0, :])
        yield r0
        def r1():
            nc.vector.reciprocal_approx_fast(
                out=recip[:, 1, :], in_=sums_all[:, g, 1, :])
        yield r1
        recip_d = p_dram.tile([NPAIR, 2, QW], F32, tag="recipd", name="recip_d")
        def store():
            # rows {0,32,64,96} hold the per-pair sums (SBUF-source
            # partition-broadcast DMA is rejected, so round-trip via DRAM)
            nc.sync.dma_start(
                out=recip_d,
                in_=recip.rearrange("(a b) h f -> a b h f", b=32)[:, 0, :, :])
        yield store
        for pair in range(NPAIR):
            rb = p_sm.tile([P, 2, QW], F32, tag="rb", bufs=4, name="rb")
            def load(rb=rb, pair=pair):
                nc.sync.dma_start(
                    out=rb,
                    in_=recip_d[pair].unsqueeze(0).partition_broadcast(P))
            yield load
            for h2 in range(2):
                def mul(rb=rb, pair=pair, h2=h2, g=g):
                    po = h2 * HD
                    sl = attnT[po:po + HD, pair, ts(g, QW)]
                    nc.vector.tensor_mul(
                        out=sl, in0=sl, in1=rb[po:po + HD, h2, :])
                yield mul

    # ---- attention unit machinery ----
    def unit_list(g):
        """(kc, width, qoff) for query group g; kc < 4g are full blocks,
        the 4 diagonal blocks shrink 512/384/256/128."""
        units = [(kc, QW, 0) for kc in range(NQB * g)]
        for j in range(NQB):
            units.append((NQB * g + j, QW - j * P, j * P))
        return units

    def emit_s(g, pair, kc, w, qoff):
        pst = p_st.tile([P, 2, QW], F32, tag="st", name="pst")
        q0 = g * QW + qoff
        nc.tensor.matmul(
            pst[:, 0, 0:w],
            lhsT=kts[0:HD, pair, ts(kc, P)],
            rhs=qts[0:HD, pair, ds(q0, w)],
            start=True, stop=True,
        )
        nc.tensor.matmul(
            pst[:, 1, 0:w],
            lhsT=kts[HD:P, pair, ts(kc, P)],
            rhs=qts[HD:P, pair, ds(q0, w)],
            start=True, stop=True,
        )
        pt = p_pt.tile([P, 2, QW], BF16, tag="pt", name="pt")
        nc.scalar.activation(
            out=pt[:, :, 0:w], in_=pst[:, :, 0:w], func=EXP, scale=0.125)
        if qoff or w < QW or kc == NQB * g:  # diagonal block: triangle mask
            nc.vector.tensor_mul(pt[:, 0, 0:P], pt[:, 0, 0:P], maskt)
            nc.vector.tensor_mul(pt[:, 1, 0:P], pt[:, 1, 0:P], maskt)
        return pt

    def emit_pv(g, pair, kc, w, qoff, pt, pvps, nkc):
        for h2 in range(2):
            nc.tensor.matmul(
                pvps[h2][:, qoff:QW],
                lhsT=vplus[:, kc, 2 * pair + h2, :],
                rhs=pt[:, h2, 0:w],
                start=(kc == 0), stop=(kc == nkc - 1),
            )

    def evict(g, pair, pvps):
        for h2 in range(2):
            po = h2 * HD
            nc.vector.tensor_copy(
                out=attnT[po:po + HD, pair, ts(g, QW)], in_=pvps[h2][0:HD, :])
            nc.vector.tensor_copy(
                out=sums_all[32 * pair:32 * pair + 1, g, h2, :],
                in_=pvps[h2][HD:HD + 1, :])

    # ---- main schedule ----
    # filler queue: closures pulled between attention units
    filler = []

    def pull(k):
        for _ in range(k):
            if not filler:
                return
            filler.pop(0)()

    def drain():
        pull(len(filler))

    # startup: token strip 0 of Q/K/V must land before group 0
    for f in proj_steps(0):
        f()

    for g in range(NTS):
        if g > 0:
            filler.extend(norm_steps(g - 1))
        if g < NTS - 1:
            filler.extend(proj_steps(g + 1))
        if g > 0:
            filler.extend(o_steps(g - 1))
        units = unit_list(g)
        npull = (4, 3, 2, 1)[g]
        for pair in range(NPAIR):
            nkc = len(units)
            pvps = [
                p_pv.tile([HD + 1, QW], F32, tag="pvA", name="pvA"),
                p_pv.tile([HD + 1, QW], F32, tag="pvB", name="pvB"),
            ]
            prev = None
            for i, (kc, w, qoff) in enumerate(units):
                pt = emit_s(g, pair, kc, w, qoff)
                pull(npull)
                if prev is not None:
                    emit_pv(g, pair, *prev)
                prev = (kc, w, qoff, pt, pvps, nkc)
            emit_pv(g, pair, *prev)
            evict(g, pair, pvps)
        # make sure next group's inputs + this group's deferred work land
        drain()

    # tail: normalization + O for the last query strip
    for f in norm_steps(NTS - 1):
        f()
    for f in o_steps(NTS - 1):
        f()


_emit_wrapped = with_exitstack(_emit)

_NC_CACHE = None


def _build():
    global _NC_CACHE
    if _NC_CACHE is not None:
        return _NC_CACHE
    nc = bacc.Bacc("TRN2", target_bir_lowering=False, debug=False)
    xT = nc.dram_tensor("xt", [D, N], BF16, kind="ExternalInput").ap()
    wq = nc.dram_tensor("wq", [D, DC], BF16, kind="ExternalInput").ap()
    wk = nc.dram_tensor("wk", [D, DC], BF16, kind="ExternalInput").ap()
    wv = nc.dram_tensor("wv", [D, DC], BF16, kind="ExternalInput").ap()
    wo = nc.dram_tensor("wo", [DC, D], BF16, kind="ExternalInput").ap()
    bq = nc.dram_tensor("bq", [DC], F32, kind="ExternalInput").ap()
    bk = nc.dram_tensor("bk", [DC], F32, kind="ExternalInput").ap()
    bv = nc.dram_tensor("bv", [DC], F32, kind="ExternalInput").ap()
    masks = nc.dram_tensor("masks", [P, P], BF16, kind="ExternalInput").ap()
    out = nc.dram_tensor("out", [N, D], F32, kind="ExternalOutput").ap()
    with tile.TileContext(nc) as tc:
        _emit_wrapped(tc, xT, wq, wk, wv, wo, bq, bk, bv, masks, out)
    nc.compile()
    _NC_CACHE = nc
    return nc


def _make_masks():
    # triangular 0/1 tile for the diagonal blocks of S^T: key <= query kept
    return np.triu(np.ones((P, P), np.float32)).astype(ml_dtypes.bfloat16)


def _in_maps(x, Wq, bq, Wk, bk, Wv, bv, Wo):
    masks = _make_masks()
    maps = []
    for b in range(B):
        xt_b = np.ascontiguousarray(np.asarray(x[b]).T)
        for g in range(GROUPS):
            sl = slice(g * DC, (g + 1) * DC)
            bf = ml_dtypes.bfloat16
            maps.append({
                "xt": xt_b.astype(bf),
                "wq": np.ascontiguousarray(Wq[:, sl]).astype(bf),
                "wk": np.ascontiguousarray(Wk[:, sl]).astype(bf),
                "wv": np.ascontiguousarray(Wv[:, sl]).astype(bf),
                "wo": np.ascontiguousarray(Wo[sl, :]).astype(bf),
                "bq": np.ascontiguousarray(bq[sl]),
                "bk": np.ascontiguousarray(bk[sl]),
                "bv": np.ascontiguousarray(bv[sl]),
                "masks": masks,
            })
    return maps


def run(inputs, trace=False, tmpdir=None):
    """Build+run on 8 cores. Returns (out [B,N,D] f32, BassKernelResults)."""
    x = np.asarray(inputs["x"], np.float32)
    args = [np.asarray(inputs[k], np.float32) for k in
            ("Wq", "bq", "Wk", "bk", "Wv", "bv", "Wo")]
    bo = np.asarray(inputs["bo"], np.float32)
    nc = _build()
    maps = _in_maps(x, *args)
    if trace:
        bass_utils.upload_artifacts = lambda d: d
    res = bass_utils.run_bass_kernel_spmd(
        nc, maps, core_ids=list(range(8)), trace=trace, tmpdir=tmpdir)
    out = np.empty((B, N, D), np.float32)
    for b in range(B):
        out[b] = res.results[2 * b]["out"] + res.results[2 * b + 1]["out"] + bo
    return out, res


def kernel(**inputs):
    out, _ = run(inputs)
    return out


# revision 7
# speedup vs baseline: 1.2428x; 1.1211x over previous
"""Multi-head causal self-attention (B=4, N=2048, D=1024, H=16) on 8 TRN2 cores.

Sharding: 8 cores = 4 batches x 2 head-groups (8 heads / 512 dims each).

v3 schedule (qs-major): the outer loop walks query strips (groups g=0..3);
for each group all 4 head-pairs run their S^T/exp/PV units. The two heads of
a pair live on SBUF partitions 0-63 / 64-127, so their S^T matmuls (K=64
contraction) are emitted back-to-back and execute CONCURRENTLY in the two
row-halves of the PE array (row tiling, tile_position (0,0)/(64,0) inferred
from base partitions). The attention stream is scalar-engine(exp)-bound, so
PE filler work is interleaved between units at an adaptive rate:
  - QKV projection chunks for token strip g+1 (needed by group g+1),
  - O-projection chunks for query strip g-1,
  - per-pair softmax normalization (emitted as soon as a pair finishes).
Softmax denominators ride as a ones-column in V (PV row HD); reciprocals are
computed with the custom-DVE reciprocal_approx_fast straight out of PSUM and
broadcast across partitions with the GPSIMD partition_broadcast (no ACT
table switches - the scalar engine runs Exp only - and no DRAM round-trip).
Causal masking = skip blocks above the diagonal; the four diagonal-region
blocks per (pair, group) shrink 512/384/256/128 wide, exp'd at exact width,
with a precomputed 0/1 triangle multiplied onto the leading 128 columns.

Dtypes: scores fp32 PSUM -> exp -> bf16 P^T; V/P^T path bf16; attnT bf16;
O-partials stored bf16 (summed f32 host-side with the other head-group).
"""

import numpy as np
import ml_dtypes

import concourse.bass as bass
import concourse.tile as tile
from concourse import bacc, mybir
from concourse import bass_utils
from concourse._compat import with_exitstack
from concourse.bass import ts, ds

B, N, D, H, HD = 4, 2048, 1024, 16, 64
GROUPS = 2              # head groups (cores per batch)
DC = D // GROUPS        # 512 dims per core
HPC = H // GROUPS       # 8 heads per core
P = 128
QW = 512                # query strip width / matmul free dim
NDIN = D // P           # 8 contraction chunks for QKV
NPAIR = DC // P         # 4 head-pairs (dq strips) per core
NTT = N // P            # 16 token tiles
NTS = N // QW           # 4 token strips == query groups
NQB = QW // P           # 4 key blocks per token strip

F32 = mybir.dt.float32
BF16 = mybir.dt.bfloat16


def _emit(ctx, tc, xT, wq, wk, wv, wo, bq, bk, bv, masks, out):
    nc = tc.nc
    EXP = mybir.ActivationFunctionType.Exp

    const = ctx.enter_context(tc.tile_pool(name="const", bufs=1))
    p_mm = ctx.enter_context(tc.tile_pool(name="p_mm", bufs=2, space="PSUM"))
    p_st = ctx.enter_context(tc.tile_pool(name="p_st", bufs=2, space="PSUM"))
    p_pv = ctx.enter_context(tc.tile_pool(name="p_pv", bufs=1, space="PSUM"))
    p_pt = ctx.enter_context(tc.tile_pool(name="p_pt", bufs=3))
    p_sm = ctx.enter_context(tc.tile_pool(name="p_sm", bufs=2))
    p_osb = ctx.enter_context(tc.tile_pool(name="p_osb", bufs=3))

    # ---- weights / constants, need-ordered on the gpsimd (SWDGE) queue ----
    # batched one-DMA-per-tensor-slice loads to minimize queue occupancy
    wqp = wq.rearrange("(c p) f -> p c f", p=P)
    wkp = wk.rearrange("(c p) f -> p c f", p=P)
    wqs = const.tile([P, NPAIR, NDIN, P], BF16)
    wks = const.tile([P, NPAIR, NDIN, P], BF16)
    nc.gpsimd.dma_start(out=wqs[:, 0], in_=wqp[:, :, ts(0, P)])
    nc.gpsimd.dma_start(out=wks[:, 0], in_=wkp[:, :, ts(0, P)])
    maskt = const.tile([P, P], BF16)
    nc.gpsimd.dma_start(out=maskt, in_=masks)
    bqt = const.tile([P, NPAIR], F32)
    nc.gpsimd.dma_start(out=bqt, in_=bq.rearrange("(s p) -> p s", p=P))
    bkt = const.tile([P, NPAIR], F32)
    nc.gpsimd.dma_start(out=bkt, in_=bk.rearrange("(s p) -> p s", p=P))
    wvt = const.tile([P, NDIN, DC], BF16)
    nc.gpsimd.dma_start(out=wvt, in_=wv.rearrange("(c p) f -> p c f", p=P))
    bvb = const.tile([P, DC], F32)
    nc.gpsimd.dma_start(out=bvb, in_=bv.unsqueeze(0).partition_broadcast(P))
    for s in range(1, NPAIR):
        nc.gpsimd.dma_start(out=wqs[:, s], in_=wqp[:, :, ts(s, P)])
        nc.gpsimd.dma_start(out=wks[:, s], in_=wkp[:, :, ts(s, P)])
    wot = const.tile([P, NPAIR, D], BF16)
    nc.gpsimd.dma_start(out=wot, in_=wo.rearrange("(c p) f -> p c f", p=P))

    # x^T resident, token-strip-major loads (sync/HWDGE queue)
    xTp = xT.rearrange("(c p) n -> p c n", p=P)
    xt = const.tile([P, NDIN, N], BF16)
    for t in range(NTS):
        nc.sync.dma_start(out=xt[:, :, ts(t, QW)], in_=xTp[:, :, ts(t, QW)])

    attnT = const.tile([P, NPAIR, N], BF16)           # unnormalized attn^T
    vplus = const.tile([P, NTT, HPC, HD + 1], BF16)   # V | ones column
    # memset on a bf16 matmul-input tile is invalid ISA; write the ones
    # column via a DVE copy from an f32 staging tile
    ones_f32 = const.tile([P, NTT * HPC], F32)
    nc.vector.memset(ones_f32, 1.0)
    nc.vector.tensor_copy(
        out=vplus[:, :, :, HD:HD + 1],
        in_=ones_f32.rearrange("p (a b) -> p a b", b=HPC).unsqueeze(3),
    )

    qts = const.tile([P, NPAIR, N], BF16)
    kts = const.tile([P, NPAIR, N], BF16)

    # ---- PE filler generators ----
    def proj_qk_steps(t, s):
        """Q and K projections of token strip t for pair strip s."""
        for which in range(2):  # 0 = Q, 1 = K
            wtile = wqs if which == 0 else wks
            btile = bqt if which == 0 else bkt
            dst = qts if which == 0 else kts
            ps = p_mm.tile([P, QW], F32, tag="mm", name="ps_proj")
            for c0 in range(0, NDIN, 2):
                def step(ps=ps, s=s, c0=c0, wtile=wtile, t=t):
                    for c in (c0, c0 + 1):
                        nc.tensor.matmul(
                            ps, lhsT=wtile[:, s, c, :],
                            rhs=xt[:, c, ts(t, QW)],
                            start=(c == 0), stop=(c == NDIN - 1),
                        )
                yield step
            def fin(ps=ps, s=s, t=t, btile=btile, dst=dst):
                nc.vector.tensor_scalar_add(
                    out=dst[:, s, ts(t, QW)], in0=ps,
                    scalar1=btile[:, s:s + 1])
            yield fin

    def proj_v_steps(t):
        for tt in range(NQB * t, NQB * (t + 1)):
            psv = p_mm.tile([P, DC], F32, tag="mm", name="psv")
            for c0 in range(0, NDIN, 2):
                def step(psv=psv, tt=tt, c0=c0):
                    for c in (c0, c0 + 1):
                        nc.tensor.matmul(
                            psv, lhsT=xt[:, c, ts(tt, P)], rhs=wvt[:, c, :],
                            start=(c == 0), stop=(c == NDIN - 1),
                        )
                yield step
            def finv(psv=psv, tt=tt):
                nc.vector.tensor_add(
                    out=vplus[:, tt, :, 0:HD],
                    in0=psv.rearrange("p (h d) -> p h d", d=HD),
                    in1=bvb.rearrange("p (h d) -> p h d", d=HD),
                )
            yield finv

    def o_steps(b):
        """O-projection for query strip b (requires strip-b attnT normed)."""
        for tt in range(NQB * b, NQB * (b + 1)):
            osb = p_osb.tile([P, D], BF16, tag="osb", name="osb")
            for half in range(2):
                pso = p_mm.tile([P, QW], F32, tag="mm", name="pso")
                for c0 in range(0, NPAIR, 2):
                    def step(pso=pso, tt=tt, half=half, c0=c0):
                        for c in (c0, c0 + 1):
                            nc.tensor.matmul(
                                pso, lhsT=attnT[:, c, ts(tt, P)],
                                rhs=wot[:, c, ds(half * QW, QW)],
                                start=(c == 0), stop=(c == NPAIR - 1),
                            )
                    yield step
                def fino(pso=pso, osb=osb, tt=tt, half=half):
                    nc.vector.tensor_copy(
                        out=osb[:, ds(half * QW, QW)], in_=pso)
                    if half == 1:
                        nc.sync.dma_start(out=out[ts(tt, P), :], in_=osb)
                yield fino

    def norm_steps(g, pair, stg):
        """Softmax normalization of (group g, pair): reciprocal of the PV
        ones-row (staged to partition 0 by evict), partition-broadcast
        (both require base partition 0), then scale attnT."""
        recip = p_sm.tile([P, 2, QW], F32, tag="recip", name="recip")
        def rstep(recip=recip, stg=stg):
            nc.vector.reciprocal_approx_fast(
                out=recip[0:1, :, :], in_=stg[0:1, :, :])
        yield rstep
        rb = p_sm.tile([P, 2, QW], F32, tag="rb", bufs=3, name="rb")
        def bstep(rb=rb, recip=recip):
            nc.gpsimd.partition_broadcast(rb, recip[0:1, :, :])
        yield bstep
        for h2 in range(2):
            def mul(rb=rb, pair=pair, h2=h2, g=g):
                po = h2 * HD
                sl = attnT[po:po + HD, pair, ts(g, QW)]
                nc.vector.tensor_mul(out=sl, in0=sl, in1=rb[po:po + HD, h2, :])
            yield mul

    # ---- attention unit machinery ----
    def unit_list(g):
        units = [(kc, QW, 0) for kc in range(NQB * g)]
        for j in range(NQB):
            units.append((NQB * g + j, QW - j * P, j * P))
        return units

    def emit_s(g, pair, kc, w, qoff):
        pst = p_st.tile([P, 2, QW], F32, tag="st", name="pst")
        q0 = g * QW + qoff
        nc.tensor.matmul(
            pst[:, 0, 0:w],
            lhsT=kts[0:HD, pair, ts(kc, P)],
            rhs=qts[0:HD, pair, ds(q0, w)],
            start=True, stop=True,
        )
        nc.tensor.matmul(
            pst[:, 1, 0:w],
            lhsT=kts[HD:P, pair, ts(kc, P)],
            rhs=qts[HD:P, pair, ds(q0, w)],
            start=True, stop=True,
        )
        pt = p_pt.tile([P, 2, QW], BF16, tag="pt", name="pt")
        nc.scalar.activation(
            out=pt[:, :, 0:w], in_=pst[:, :, 0:w], func=EXP, scale=0.125)
        if kc >= NQB * g:  # diagonal block: triangle mask on leading 128
            nc.vector.tensor_mul(pt[:, 0, 0:P], pt[:, 0, 0:P], maskt)
            nc.vector.tensor_mul(pt[:, 1, 0:P], pt[:, 1, 0:P], maskt)
        return pt

    def emit_pv(g, pair, kc, w, qoff, pt, pvps, nkc):
        for h2 in range(2):
            nc.tensor.matmul(
                pvps[h2][:, qoff:QW],
                lhsT=vplus[:, kc, 2 * pair + h2, :],
                rhs=pt[:, h2, 0:w],
                start=(kc == 0), stop=(kc == nkc - 1),
            )

    def evict(g, pair, pvps):
        """PSUM -> SBUF: attn values to attnT, ones-row (denominators) to
        partition 0 of a staging tile (releases the pvp banks promptly)."""
        stg = p_sm.tile([P, 2, QW], F32, tag="stg", name="stg")
        for h2 in range(2):
            po = h2 * HD
            nc.vector.tensor_copy(
                out=attnT[po:po + HD, pair, ts(g, QW)], in_=pvps[h2][0:HD, :])
            nc.vector.tensor_copy(
                out=stg[0:1, h2, :], in_=pvps[h2][HD:HD + 1, :])
        return stg

    # ---- adaptive filler queue ----
    filler = []
    acc = [0.0]

    def pull_units(remaining):
        """Pull an even share of queued filler work for one unit slot."""
        if remaining <= 0:
            k = len(filler)
        else:
            acc[0] += len(filler) / remaining
            k = int(acc[0])
            acc[0] -= k
        for _ in range(min(k, len(filler))):
            filler.pop(0)()

    def drain():
        while filler:
            filler.pop(0)()

    # ---- main schedule ----
    # startup: V of token strip 0 + pair-0 Q/K land first; remaining pairs'
    # Q/K become filler so group-0 attention starts ASAP
    for f in proj_v_steps(0):
        f()
    for f in proj_qk_steps(0, 0):
        f()
    for s in range(1, NPAIR):
        filler.extend(proj_qk_steps(0, s))

    for g in range(NTS):
        if g < NTS - 1:
            for s in range(NPAIR):
                filler.extend(proj_qk_steps(g + 1, s))
            filler.extend(proj_v_steps(g + 1))
        if g > 0:
            filler.extend(o_steps(g - 1))
        units = unit_list(g)
        nkc = len(units)
        rem_units = NPAIR * (nkc + 1)
        for pair in range(NPAIR):
            pvps = [
                p_pv.tile([HD + 1, QW], F32, tag="pvA", name="pvA"),
                p_pv.tile([HD + 1, QW], F32, tag="pvB", name="pvB"),
            ]
            prev = None
            for (kc, w, qoff) in units:
                pt = emit_s(g, pair, kc, w, qoff)
                pull_units(rem_units)
                rem_units -= 1
                if prev is not None:
                    emit_pv(g, pair, *prev)
                prev = (kc, w, qoff, pt, pvps, nkc)
            emit_pv(g, pair, *prev)
            stg = evict(g, pair, pvps)
            filler.extend(norm_steps(g, pair, stg))
            pull_units(rem_units)
            rem_units -= 1
        if g < NTS - 1:
            # next group needs its projections landed; drain leftovers
            drain()

    # tail: remaining normalization + O for the last query strip
    drain()
    for f in o_steps(NTS - 1):
        f()


_emit_wrapped = with_exitstack(_emit)

_NC_CACHE = None


def _build():
    global _NC_CACHE
    if _NC_CACHE is not None:
        return _NC_CACHE
    nc = bacc.Bacc("TRN2", target_bir_lowering=False, debug=False)
    xT = nc.dram_tensor("xt", [D, N], BF16, kind="ExternalInput").ap()
    wq = nc.dram_tensor("wq", [D, DC], BF16, kind="ExternalInput").ap()
    wk = nc.dram_tensor("wk", [D, DC], BF16, kind="ExternalInput").ap()
    wv = nc.dram_tensor("wv", [D, DC], BF16, kind="ExternalInput").ap()
    wo = nc.dram_tensor("wo", [DC, D], BF16, kind="ExternalInput").ap()
    bq = nc.dram_tensor("bq", [DC], F32, kind="ExternalInput").ap()
    bk = nc.dram_tensor("bk", [DC], F32, kind="ExternalInput").ap()
    bv = nc.dram_tensor("bv", [DC], F32, kind="ExternalInput").ap()
    masks = nc.dram_tensor("masks", [P, P], BF16, kind="ExternalInput").ap()
    out = nc.dram_tensor("out", [N, D], BF16, kind="ExternalOutput").ap()
    with tile.TileContext(nc) as tc:
        _emit_wrapped(tc, xT, wq, wk, wv, wo, bq, bk, bv, masks, out)
    nc.compile()
    _NC_CACHE = nc
    return nc


def _make_masks():
    # triangular 0/1 tile for the diagonal blocks of S^T: key <= query kept
    return np.triu(np.ones((P, P), np.float32)).astype(ml_dtypes.bfloat16)


def _in_maps(x, Wq, bq, Wk, bk, Wv, bv, Wo):
    masks = _make_masks()
    maps = []
    for b in range(B):
        xt_b = np.ascontiguousarray(np.asarray(x[b]).T)
        for g in range(GROUPS):
            sl = slice(g * DC, (g + 1) * DC)
            bf = ml_dtypes.bfloat16
            maps.append({
                "xt": xt_b.astype(bf),
                "wq": np.ascontiguousarray(Wq[:, sl]).astype(bf),
                "wk": np.ascontiguousarray(Wk[:, sl]).astype(bf),
                "wv": np.ascontiguousarray(Wv[:, sl]).astype(bf),
                "wo": np.ascontiguousarray(Wo[sl, :]).astype(bf),
                "bq": np.ascontiguousarray(bq[sl]),
                "bk": np.ascontiguousarray(bk[sl]),
                "bv": np.ascontiguousarray(bv[sl]),
                "masks": masks,
            })
    return maps


def run(inputs, trace=False, tmpdir=None):
    """Build+run on 8 cores. Returns (out [B,N,D] f32, BassKernelResults)."""
    x = np.asarray(inputs["x"], np.float32)
    args = [np.asarray(inputs[k], np.float32) for k in
            ("Wq", "bq", "Wk", "bk", "Wv", "bv", "Wo")]
    bo = np.asarray(inputs["bo"], np.float32)
    nc = _build()
    maps = _in_maps(x, *args)
    if trace:
        bass_utils.upload_artifacts = lambda d: d
    res = bass_utils.run_bass_kernel_spmd(
        nc, maps, core_ids=list(range(8)), trace=trace, tmpdir=tmpdir)
    out = np.empty((B, N, D), np.float32)
    for b in range(B):
        out[b] = (res.results[2 * b]["out"].astype(np.float32)
                  + res.results[2 * b + 1]["out"].astype(np.float32) + bo)
    return out, res


def kernel(**inputs):
    out, _ = run(inputs)
    return out
